# revision 10
# baseline (speedup 1.0000x reference)
"""Trainium2 Bass kernel for AttnDecoderRNN single-step forward.

Sharding (8 NeuronCores, tensor-parallel):
  - attn_W / attn_b / encoder_outputs sharded over seq_len (4096 -> 512/core)
  - GRU gates sharded over hidden (1024 -> 128/core, 384 gate rows/core)
  - out_W / out_b sharded over vocab (50257 -> pad 51200 -> 6400/core)
  - combine (1324->300) replicated on every core

All matvecs run on the TensorEngine with M=1 (batch) and the big weight
matrix as the *streaming* rhs operand, so weight bytes flow through the PE
at 1 col/cycle with only a 1-column LDWEIGHTS.  Biases are folded in as a
K=1 matmul with a ones lhsT.  Softmax / log-softmax use the online
max/sumexp merge: each core AllGathers (neg-max, sumexp) pairs (8 floats)
and applies the correction locally, so only tiny collectives sit on the
critical path.  Free-dim -> partition-dim relayouts go through small
SBUF->SBUF DMAs (PSUM banks stay disjoint; every PE instruction needs at
most 2 semaphore waits, which the MM instruction encoding requires).
"""

import sys

sys.path.insert(0, "/opt/trn_rl_repo")

import numpy as np

import concourse.bass as bass
import concourse.mybir as mybir
from concourse import bacc, tile

F32 = mybir.dt.float32
F32R = mybir.dt.float32r
BF16 = mybir.dt.bfloat16

NCORES = 8
I_SZ, H_SZ, V_SZ, L_SZ = 300, 1024, 50257, 4096
LSH = L_SZ // NCORES        # 512 seq positions per core
KA_T = 11                   # attn contraction tiles (11*128 = 1408 >= 1324)
VI = 6400                   # vocab shard per core (padded)
VPAD = VI * NCORES          # 51200
NCH = 13                    # stage-D chunks per core
CHS = [512] * 12 + [256]    # chunk widths (sum = 6400)
GS = 384                    # gate rows per core (3*128)
HS = H_SZ // NCORES         # 128 hidden units per core
NEG_BIG = -1.0e30
RG = [list(range(NCORES))]

MODE = "bf16"  # one of: f32, f32r, mixed, bf16


def _dtypes(mode):
    if mode == "bf16":
        return BF16, BF16
    if mode == "mixed":
        return F32, BF16
    return F32, F32


def build(mode=MODE):
    """Build the SPMD Bass program (same program on all 8 cores)."""
    adt, odt = _dtypes(mode)           # storage dtype: attn/gru weights, out_W
    use_f32r = mode in ("f32r", "mixed")

    def WA(ap):  # matmul-operand wrapper for the attn/gru path
        return ap.bitcast(F32R) if (use_f32r and adt == F32) else ap

    def WO(ap):  # matmul-operand wrapper for the output-projection path
        return ap.bitcast(F32R) if (use_f32r and odt == F32) else ap

    def cast_dma(out, in_, casting):
        # small relayout/cast DMAs go through SWDGE (gpsimd): the HWDGE
        # direct-DMA encoding only fits a single sync wait
        nc.gpsimd.dma_start(out=out, in_=in_)

    nc = bacc.Bacc("TRN2", target_bir_lowering=False, debug=False,
                   num_devices=NCORES)

    # ---------------- external inputs (host pre-shaped / pre-permuted) ----
    d_vattn = nc.dram_tensor("v_attn", [128, KA_T], adt, kind="ExternalInput")
    d_aw = nc.dram_tensor("aw", [128, KA_T, LSH], adt, kind="ExternalInput")
    d_ab = nc.dram_tensor("ab", [LSH], adt, kind="ExternalInput")
    d_enc = nc.dram_tensor("enc", [128, 4, H_SZ], adt, kind="ExternalInput")
    d_emb = nc.dram_tensor("embk", [128, 3], adt, kind="ExternalInput")
    d_cw = nc.dram_tensor("cw", [128, KA_T, I_SZ], adt, kind="ExternalInput")
    d_cb = nc.dram_tensor("cb", [I_SZ], adt, kind="ExternalInput")
    d_h = nc.dram_tensor("h_full", [128, 8], adt, kind="ExternalInput")
    d_wih = nc.dram_tensor("wih", [128, 3, GS], adt, kind="ExternalInput")
    d_whh = nc.dram_tensor("whh", [128, 8, GS], adt, kind="ExternalInput")
    d_bih = nc.dram_tensor("bih", [GS], adt, kind="ExternalInput")
    d_bhh = nc.dram_tensor("bhh", [GS], adt, kind="ExternalInput")
    d_hmy = nc.dram_tensor("hmy", [HS], F32, kind="ExternalInput")
    d_wout_a = nc.dram_tensor("wout_a", [12, 128, 8, 512], odt,
                              kind="ExternalInput")
    d_wout_b = nc.dram_tensor("wout_b", [128, 8, 256], odt,
                              kind="ExternalInput")
    d_outb = nc.dram_tensor("outb", [VI], odt, kind="ExternalInput")

    # ---------------- external outputs ------------------------------------
    d_out_lp = nc.dram_tensor("out_logp", [NCH, 512], F32,
                              kind="ExternalOutput")
    d_out_h = nc.dram_tensor("out_h", [HS], F32, kind="ExternalOutput")
    d_out_aw = nc.dram_tensor("out_attnw", [LSH], F32, kind="ExternalOutput")

    X = mybir.AxisListType.X
    ADD = mybir.AluOpType.add
    MIN = mybir.AluOpType.min
    AF = mybir.ActivationFunctionType

    with tile.TileContext(nc) as tc:
        with (
            tc.tile_pool(name="w", bufs=1) as wp,
            tc.tile_pool(name="wout", bufs=4) as wop,
            tc.tile_pool(name="ch", bufs=2) as chp,
            tc.tile_pool(name="ps", bufs=1, space="PSUM") as pp,
            tc.tile_pool(name="dram", bufs=1, space="DRAM") as dp,
        ):
            # ---- collective bounce buffers in DRAM ----
            cc_a_in = dp.tile([8], F32, name="cc_a_in")
            cc_a_out = dp.tile([8 * NCORES], F32, addr_space="Shared",
                               name="cc_a_out")
            cc_b_in = dp.tile([H_SZ], F32, name="cc_b_in")
            cc_b_out = dp.tile([H_SZ], F32, addr_space="Shared",
                               name="cc_b_out")
            cc_h_in = dp.tile([HS], F32, name="cc_h_in")
            cc_h_out = dp.tile([H_SZ], F32, addr_space="Shared",
                               name="cc_h_out")
            cc_d_in = dp.tile([8], F32, name="cc_d_in")
            cc_d_out = dp.tile([8 * NCORES], F32, addr_space="Shared",
                               name="cc_d_out")

            # ---- constants ----
            ones_a = wp.tile([1, 1], adt, name="ones_a")
            nc.vector.memset(ones_a[:, :], 1.0)
            ones_o = wp.tile([1, 1], odt, name="ones_o")
            nc.vector.memset(ones_o[:, :], 1.0)
            ones13 = wp.tile([1, NCH], F32, name="ones13")
            nc.vector.memset(ones13[:, :], 1.0)
            logits = wp.tile([NCH, 512], F32, name="logits")
            nc.vector.memset(logits[:, :], NEG_BIG)
            x_sb = wp.tile([1, 384], F32, name="x_sb")
            nc.vector.memset(x_sb[:, :], 0.0)

            # ---- weight / vector loads (priority order) ----
            vk = wp.tile([128, KA_T], adt, name="vk")
            nc.sync.dma_start(out=vk[:, :], in_=d_vattn[:, :])
            aw_sb = wp.tile([128, KA_T, LSH], adt, name="aw_sb")
            nc.sync.dma_start(out=aw_sb[:, :, :], in_=d_aw[:, :, :])
            ab_sb = wp.tile([1, LSH], adt, name="ab_sb")
            nc.sync.dma_start(out=ab_sb[0:1, :], in_=d_ab[:])
            enc_sb = wp.tile([128, 4, H_SZ], adt, name="enc_sb")
            nc.sync.dma_start(out=enc_sb[:, :, :], in_=d_enc[:, :, :])
            hk = wp.tile([128, 8], adt, name="hk")
            nc.sync.dma_start(out=hk[:, :], in_=d_h[:, :])
            whh_sb = wp.tile([128, 8, GS], adt, name="whh_sb")
            nc.sync.dma_start(out=whh_sb[:, :, :], in_=d_whh[:, :, :])
            bhh_sb = wp.tile([1, GS], adt, name="bhh_sb")
            nc.sync.dma_start(out=bhh_sb[0:1, :], in_=d_bhh[:])
            comb_e = wp.tile([128, 3], adt, name="comb_e")
            nc.sync.dma_start(out=comb_e[:, :], in_=d_emb[:, :])
            cw_sb = wp.tile([128, KA_T, I_SZ], adt, name="cw_sb")
            nc.sync.dma_start(out=cw_sb[:, :, :], in_=d_cw[:, :, :])
            cb_sb = wp.tile([1, I_SZ], adt, name="cb_sb")
            nc.sync.dma_start(out=cb_sb[0:1, :], in_=d_cb[:])
            wih_sb = wp.tile([128, 3, GS], adt, name="wih_sb")
            nc.sync.dma_start(out=wih_sb[:, :, :], in_=d_wih[:, :, :])
            bih_sb = wp.tile([1, GS], adt, name="bih_sb")
            nc.sync.dma_start(out=bih_sb[0:1, :], in_=d_bih[:])
            hmy_sb = wp.tile([1, HS], F32, name="hmy_sb")
            nc.sync.dma_start(out=hmy_sb[0:1, :], in_=d_hmy[:])
            outb_sb = wp.tile([1, VI], odt, name="outb_sb")
            nc.sync.dma_start(out=outb_sb[0:1, :], in_=d_outb[:])

            # ============= stage A: attention scores + local stats ========
            ps_sc = pp.tile([1, LSH], F32, name="ps_sc")
            for t in range(KA_T):
                nc.tensor.matmul(ps_sc[0:1, :], WA(vk[:, t:t + 1]),
                                 WA(aw_sb[:, t, :]),
                                 start=(t == 0), stop=False)
            nc.tensor.matmul(ps_sc[0:1, :], WA(ones_a[:, :]),
                             WA(ab_sb[0:1, :]), start=False, stop=True)

            nm_a = wp.tile([1, 1], F32, name="nm_a")      # -max(scores)
            nc.vector.reduce_max(nm_a[:, :], ps_sc[0:1, :], X, negate=True)
            e_loc = wp.tile([1, LSH], F32, name="e_loc")
            s_a = wp.tile([1, 1], F32, name="s_a")
            nc.scalar.activation(e_loc[0:1, :], ps_sc[0:1, :], AF.Exp,
                                 bias=nm_a[:, :], scale=1.0,
                                 accum_out=s_a[:, :])
            ms_a = wp.tile([1, 8], F32, name="ms_a")
            nc.vector.memset(ms_a[:, :], 0.0)
            nc.vector.tensor_copy(ms_a[0:1, 0:1], nm_a[:, :])
            nc.vector.tensor_copy(ms_a[0:1, 1:2], s_a[:, :])
            nc.gpsimd.dma_start(out=cc_a_in[:], in_=ms_a[0:1, :])
            nc.gpsimd.collective_compute(
                "AllGather", mybir.AluOpType.bypass, replica_groups=RG,
                ins=[cc_a_in[:]], outs=[cc_a_out[:]])

            # ---- relayout e_loc to partition layout (overlaps AG#1) ------
            ek = wp.tile([128, 4], adt, name="ek")
            cast_dma(ek[:, :], e_loc[0:1, :], adt != F32)

            # ============= stage B: partial attn_applied ==================
            ps_att = pp.tile([1, H_SZ], F32, name="ps_att")
            for nb in range(2):
                sl = slice(nb * 512, (nb + 1) * 512)
                for t in range(4):
                    nc.tensor.matmul(ps_att[0:1, sl], WA(ek[:, t:t + 1]),
                                     WA(enc_sb[:, t, sl]),
                                     start=(t == 0), stop=(t == 3))

            # ---- gh = h @ W_hh.T + b_hh (PE busywork during AG/AR) -------
            ps_gh = pp.tile([1, GS], F32, name="ps_gh")
            for t in range(8):
                nc.tensor.matmul(ps_gh[0:1, :], WA(hk[:, t:t + 1]),
                                 WA(whh_sb[:, t, :]),
                                 start=(t == 0), stop=False)
            nc.tensor.matmul(ps_gh[0:1, :], WA(ones_a[:, :]),
                             WA(bhh_sb[0:1, :]), start=False, stop=True)
            gh_sb = wp.tile([1, GS], F32, name="gh_sb")
            nc.scalar.copy(gh_sb[0:1, :], ps_gh[0:1, :])

            # ---- AG#1 result: merge global max / sumexp ------------------
            ms8_a = wp.tile([1, 8 * NCORES], F32, name="ms8_a")
            nc.gpsimd.dma_start(out=ms8_a[0:1, :], in_=cc_a_out[:])
            ms8_av = ms8_a.rearrange("p (r k) -> p r k", k=8)
            nmG_a = wp.tile([1, 1], F32, name="nmG_a")   # -global max
            nc.vector.tensor_reduce(nmG_a[:, :], ms8_av[:, :, 0], X, MIN)
            corr_a = wp.tile([1, NCORES], F32, name="corr_a")
            nc.scalar.activation(corr_a[0:1, :], ms8_av[:, :, 0], AF.Exp,
                                 bias=nmG_a[:, :], scale=-1.0)
            sc_a = wp.tile([1, NCORES], F32, name="sc_a")
            nc.vector.tensor_mul(sc_a[0:1, :], corr_a[0:1, :],
                                 ms8_av[:, :, 1])
            S_a = wp.tile([1, 1], F32, name="S_a")
            nc.vector.tensor_reduce(S_a[:, :], sc_a[0:1, :], X, ADD)
            rS_a = wp.tile([1, 1], F32, name="rS_a")
            nc.vector.reciprocal(rS_a[:, :], S_a[:, :])
            cme_a = wp.tile([1, 1], F32, name="cme_a")   # exp(m_c - M)
            nc.scalar.activation(cme_a[0:1, :], nm_a[0:1, :], AF.Exp,
                                 bias=nmG_a[:, :], scale=-1.0)
            scme = wp.tile([1, 1], F32, name="scme")     # exp(m_c-M)/S
            nc.vector.tensor_mul(scme[:, :], cme_a[:, :], rS_a[:, :])

            # local attention weights output slice
            w_loc = wp.tile([1, LSH], F32, name="w_loc")
            nc.scalar.activation(w_loc[0:1, :], e_loc[0:1, :], AF.Copy,
                                 bias=0.0, scale=scme[:, :])
            nc.gpsimd.dma_start(out=d_out_aw[:], in_=w_loc[0:1, :])

            # scaled partial attn_applied -> AllReduce
            attp = wp.tile([1, H_SZ], F32, name="attp")
            nc.scalar.activation(attp[0:1, :], ps_att[0:1, :], AF.Copy,
                                 bias=0.0, scale=scme[:, :])
            nc.gpsimd.dma_start(out=cc_b_in[:], in_=attp[0:1, :])
            nc.gpsimd.collective_compute(
                "AllReduce", ADD, replica_groups=RG,
                ins=[cc_b_in[:]], outs=[cc_b_out[:]])

            # ============= stage C: combine + GRU =========================
            comb_a = wp.tile([128, 8], adt, name="comb_a")
            cast_dma(comb_a[:, :], cc_b_out[:], adt != F32)

            ps_x = pp.tile([1, I_SZ], F32, name="ps_x")
            for t in range(3):
                nc.tensor.matmul(ps_x[0:1, :], WA(comb_e[:, t:t + 1]),
                                 WA(cw_sb[:, t, :]),
                                 start=(t == 0), stop=False)
            for t in range(3, KA_T):
                nc.tensor.matmul(ps_x[0:1, :], WA(comb_a[:, t - 3:t - 2]),
                                 WA(cw_sb[:, t, :]),
                                 start=False, stop=False)
            nc.tensor.matmul(ps_x[0:1, :], WA(ones_a[:, :]),
                             WA(cb_sb[0:1, :]), start=False, stop=True)
            nc.scalar.activation(x_sb[0:1, 0:I_SZ], ps_x[0:1, :], AF.Relu)

            xk = wp.tile([128, 3], adt, name="xk")
            cast_dma(xk[:, :], x_sb[0:1, :], adt != F32)

            ps_gi = pp.tile([1, GS], F32, name="ps_gi")
            for t in range(3):
                nc.tensor.matmul(ps_gi[0:1, :], WA(xk[:, t:t + 1]),
                                 WA(wih_sb[:, t, :]),
                                 start=(t == 0), stop=False)
            nc.tensor.matmul(ps_gi[0:1, :], WA(ones_a[:, :]),
                             WA(bih_sb[0:1, :]), start=False, stop=True)

            # gates: r,z = sigmoid(gi+gh)[0:256]; n = tanh(gi_n + r*gh_n)
            rz_in = wp.tile([1, 256], F32, name="rz_in")
            nc.vector.tensor_add(rz_in[0:1, :], ps_gi[0:1, 0:256],
                                 gh_sb[0:1, 0:256])
            rz = wp.tile([1, 256], F32, name="rz")
            nc.scalar.activation(rz[0:1, :], rz_in[0:1, :], AF.Sigmoid)
            rn = wp.tile([1, HS], F32, name="rn")
            nc.vector.tensor_mul(rn[0:1, :], rz[0:1, 0:HS],
                                 gh_sb[0:1, 256:GS])
            n_in = wp.tile([1, HS], F32, name="n_in")
            nc.vector.tensor_add(n_in[0:1, :], ps_gi[0:1, 256:GS],
                                 rn[0:1, :])
            n_t = wp.tile([1, HS], F32, name="n_t")
            nc.scalar.activation(n_t[0:1, :], n_in[0:1, :], AF.Tanh)
            d_tl = wp.tile([1, HS], F32, name="d_tl")
            nc.vector.tensor_sub(d_tl[0:1, :], hmy_sb[0:1, :], n_t[0:1, :])
            zd = wp.tile([1, HS], F32, name="zd")
            nc.vector.tensor_mul(zd[0:1, :], rz[0:1, HS:256], d_tl[0:1, :])
            hn_new = wp.tile([1, HS], F32, name="hn_new")
            nc.vector.tensor_add(hn_new[0:1, :], n_t[0:1, :], zd[0:1, :])

            nc.gpsimd.dma_start(out=d_out_h[:], in_=hn_new[0:1, :])
            nc.gpsimd.dma_start(out=cc_h_in[:], in_=hn_new[0:1, :])
            nc.gpsimd.collective_compute(
                "AllGather", mybir.AluOpType.bypass, replica_groups=RG,
                ins=[cc_h_in[:]], outs=[cc_h_out[:]])

            hnk = wp.tile([128, 8], odt, name="hnk")
            cast_dma(hnk[:, :], cc_h_out[:], odt != F32)

            # ============= stage D: logits + log_softmax ==================
            for j in range(NCH):
                n_j = CHS[j]
                wt = wop.tile([128, 8, 512], odt, tag="wt", name="wt")
                if j < 12:
                    nc.sync.dma_start(out=wt[:, :, :],
                                      in_=d_wout_a[j, :, :, :])
                else:
                    nc.sync.dma_start(out=wt[:, :, 0:256],
                                      in_=d_wout_b[:, :, :])
                ps_d = pp.tile([1, 512], F32, tag="ps_d", name="ps_d",
                               bufs=2)
                for t in range(8):
                    nc.tensor.matmul(ps_d[0:1, 0:n_j],
                                     WO(hnk[:, t:t + 1]),
                                     WO(wt[:, t, 0:n_j]),
                                     start=(t == 0), stop=False)
                nc.tensor.matmul(ps_d[0:1, 0:n_j], WO(ones_o[:, :]),
                                 WO(outb_sb[0:1, j * 512:j * 512 + n_j]),
                                 start=False, stop=True)
                ch = chp.tile([1, 512], F32, tag="ch", name="ch")
                nc.scalar.copy(ch[0:1, 0:n_j], ps_d[0:1, 0:n_j])
                nc.gpsimd.dma_start(out=logits[j:j + 1, 0:n_j],
                                  in_=ch[0:1, 0:n_j])

            # ---- local log-softmax stats over [13, 512] ------------------
            nm_d = wp.tile([NCH, 1], F32, name="nm_d")   # -rowmax
            nc.vector.reduce_max(nm_d[:, :], logits[:, :], X, negate=True)
            e_d = wp.tile([NCH, 512], F32, name="e_d")
            s_d = wp.tile([NCH, 1], F32, name="s_d")
            nc.scalar.activation(e_d[:, :], logits[:, :], AF.Exp,
                                 bias=nm_d[:, :], scale=1.0,
                                 accum_out=s_d[:, :])
            mT = wp.tile([1, NCH], F32, name="mT")
            nc.gpsimd.dma_start(out=mT[0:1, :], in_=nm_d[:, 0:1])
            sT = wp.tile([1, NCH], F32, name="sT")
            nc.gpsimd.dma_start(out=sT[0:1, :], in_=s_d[:, 0:1])
            nmL = wp.tile([1, 1], F32, name="nmL")       # -local max
            nc.vector.tensor_reduce(nmL[:, :], mT[0:1, :], X, MIN)
            corr_d = wp.tile([1, NCH], F32, name="corr_d")
            nc.scalar.activation(corr_d[0:1, :], mT[0:1, :], AF.Exp,
                                 bias=nmL[:, :], scale=-1.0)
            scd = wp.tile([1, NCH], F32, name="scd")
            nc.vector.tensor_mul(scd[0:1, :], corr_d[0:1, :], sT[0:1, :])
            S_dl = wp.tile([1, 1], F32, name="S_dl")
            nc.vector.tensor_reduce(S_dl[:, :], scd[0:1, :], X, ADD)

            ms_d = wp.tile([1, 8], F32, name="ms_d")
            nc.vector.memset(ms_d[:, :], 0.0)
            nc.vector.tensor_copy(ms_d[0:1, 0:1], nmL[:, :])
            nc.vector.tensor_copy(ms_d[0:1, 1:2], S_dl[:, :])
            nc.gpsimd.dma_start(out=cc_d_in[:], in_=ms_d[0:1, :])
            nc.gpsimd.collective_compute(
                "AllGather", mybir.AluOpType.bypass, replica_groups=RG,
                ins=[cc_d_in[:]], outs=[cc_d_out[:]])

            ms8_d = wp.tile([1, 8 * NCORES], F32, name="ms8_d")
            nc.gpsimd.dma_start(out=ms8_d[0:1, :], in_=cc_d_out[:])
            ms8_dv = ms8_d.rearrange("p (r k) -> p r k", k=8)
            nmG_d = wp.tile([1, 1], F32, name="nmG_d")   # -global max
            nc.vector.tensor_reduce(nmG_d[:, :], ms8_dv[:, :, 0], X, MIN)
            corr_g = wp.tile([1, NCORES], F32, name="corr_g")
            nc.scalar.activation(corr_g[0:1, :], ms8_dv[:, :, 0], AF.Exp,
                                 bias=nmG_d[:, :], scale=-1.0)
            sc_g = wp.tile([1, NCORES], F32, name="sc_g")
            nc.vector.tensor_mul(sc_g[0:1, :], corr_g[0:1, :],
                                 ms8_dv[:, :, 1])
            S_g = wp.tile([1, 1], F32, name="S_g")
            nc.vector.tensor_reduce(S_g[:, :], sc_g[0:1, :], X, ADD)
            lnS = wp.tile([1, 1], F32, name="lnS")
            nc.scalar.activation(lnS[0:1, :], S_g[0:1, :], AF.Ln)
            nshift = wp.tile([1, 1], F32, name="nshift")  # -(M + ln S)
            nc.vector.tensor_sub(nshift[:, :], nmG_d[:, :], lnS[:, :])
            nsh13 = wp.tile([1, NCH], F32, name="nsh13")
            nc.vector.tensor_scalar_mul(nsh13[0:1, :], ones13[0:1, :],
                                        nshift[:, :])
            nb13 = wp.tile([NCH, 1], F32, name="nb13")
            nc.gpsimd.dma_start(out=nb13[:, 0:1], in_=nsh13[0:1, :])

            outlp = wp.tile([NCH, 512], F32, name="outlp")
            nc.scalar.activation(outlp[:, :], logits[:, :], AF.Identity,
                                 bias=nb13[:, :], scale=1.0)
            nc.gpsimd.dma_start(out=d_out_lp[:, :], in_=outlp[:, :])

    nc.compile()
    return nc


def prepare_in_maps(embedded, hidden, encoder_outputs, attn_W, attn_b,
                    combine_W, combine_b, W_ih, W_hh, b_ih, b_hh, out_W,
                    out_b, mode=MODE):
    adt, odt = _dtypes(mode)
    anp = mybir.dt.np(adt)
    onp = mybir.dt.np(odt)

    f32 = np.float32
    emb = np.asarray(embedded, f32).reshape(I_SZ)
    h0 = np.asarray(hidden, f32).reshape(H_SZ)
    enc = np.asarray(encoder_outputs, f32)
    aW = np.asarray(attn_W, f32)
    ab = np.asarray(attn_b, f32)
    cW = np.asarray(combine_W, f32)
    cb = np.asarray(combine_b, f32)
    Wih = np.asarray(W_ih, f32)
    Whh = np.asarray(W_hh, f32)
    bih = np.asarray(b_ih, f32)
    bhh = np.asarray(b_hh, f32)
    oW = np.asarray(out_W, f32)
    ob = np.asarray(out_b, f32)

    v = np.zeros(128 * KA_T, f32)
    v[:I_SZ] = emb
    v[I_SZ:I_SZ + H_SZ] = h0
    v_attn = v.reshape(128, KA_T).astype(anp)

    emb_pad = np.zeros(384, f32)
    emb_pad[:I_SZ] = emb
    embk = emb_pad.reshape(128, 3).astype(anp)
    h_full = h0.reshape(128, 8).astype(anp)

    AWT = aW.T  # [1324, 4096]
    CWT = cW.T  # [1324, 300]
    cw_e = np.zeros((384, I_SZ), f32)
    cw_e[:I_SZ] = CWT[:I_SZ]
    cw_host = np.concatenate(
        [cw_e.reshape(128, 3, I_SZ), CWT[I_SZ:].reshape(128, 8, I_SZ)],
        axis=1).astype(anp)  # [128, 11, 300]

    WIHT = Wih.T  # [300, 3072]
    WHHT = Whh.T  # [1024, 3072]

    WTp = np.zeros((H_SZ, VPAD), f32)
    WTp[:, :V_SZ] = oW.T
    obp = np.full(VPAD, NEG_BIG, f32)
    obp[:V_SZ] = ob

    in_maps = []
    for c in range(NCORES):
        AWc = np.zeros((128 * KA_T, LSH), f32)
        AWc[:I_SZ + H_SZ] = AWT[:, c * LSH:(c + 1) * LSH]
        gidx = np.concatenate([np.arange(c * HS, (c + 1) * HS) + k * H_SZ
                               for k in range(3)])
        wih_p = np.zeros((384, GS), f32)
        wih_p[:I_SZ] = WIHT[:, gidx]
        Wc = WTp[:, c * VI:(c + 1) * VI].reshape(128, 8, VI)
        in_maps.append({
            "v_attn": v_attn,
            "aw": AWc.reshape(128, KA_T, LSH).astype(anp),
            "ab": ab[c * LSH:(c + 1) * LSH].astype(anp),
            "enc": enc[c * LSH:(c + 1) * LSH].reshape(128, 4, H_SZ)
                   .astype(anp).copy(),
            "embk": embk,
            "cw": cw_host,
            "cb": cb.astype(anp),
            "h_full": h_full,
            "wih": wih_p.reshape(128, 3, GS).astype(anp).copy(),
            "whh": WHHT[:, gidx].reshape(128, 8, GS).astype(anp),
            "bih": bih[gidx].astype(anp),
            "bhh": bhh[gidx].astype(anp),
            "hmy": h0[c * HS:(c + 1) * HS],
            "wout_a": Wc[:, :, :6144].reshape(128, 8, 12, 512)
                      .transpose(2, 0, 1, 3).astype(onp).copy(),
            "wout_b": Wc[:, :, 6144:].astype(onp).copy(),
            "outb": obp[c * VI:(c + 1) * VI].astype(onp),
        })
    return in_maps


def gather_outputs(results):
    """results: list of 8 dicts with out_logp/out_h/out_attnw."""
    lp_parts = []
    for c in range(NCORES):
        r = np.asarray(results[c]["out_logp"], np.float32).reshape(NCH, 512)
        lp_parts.append(r[:12].reshape(-1))
        lp_parts.append(r[12, :256])
    output = np.concatenate(lp_parts)[:V_SZ][None, :]
    h_new = np.concatenate(
        [np.asarray(results[c]["out_h"], np.float32).reshape(-1)
         for c in range(NCORES)])[None, None, :]
    attn_w = np.concatenate(
        [np.asarray(results[c]["out_attnw"], np.float32).reshape(-1)
         for c in range(NCORES)])[None, :]
    return output, h_new, attn_w


_NC_CACHE = {}


def kernel(embedded, hidden, encoder_outputs, attn_W, attn_b,
           combine_W, combine_b, W_ih, W_hh, b_ih, b_hh, out_W, out_b):
    from concourse.bass_utils import run_bass_kernel_spmd

    if MODE not in _NC_CACHE:
        _NC_CACHE[MODE] = build(MODE)
    nc = _NC_CACHE[MODE]
    in_maps = prepare_in_maps(embedded, hidden, encoder_outputs, attn_W,
                              attn_b, combine_W, combine_b, W_ih, W_hh,
                              b_ih, b_hh, out_W, out_b, mode=MODE)
    res = run_bass_kernel_spmd(nc, in_maps, list(range(NCORES)))
    return gather_outputs(res.results)


# revision 11
# speedup vs baseline: 1.0191x; 1.0191x over previous
"""Trainium2 Bass kernel for AttnDecoderRNN single-step forward.

Sharding (8 NeuronCores, tensor-parallel):
  - attn_W / attn_b / encoder_outputs sharded over seq_len (4096 -> 512/core)
  - GRU gates sharded over hidden (1024 -> 128/core, 384 gate rows/core)
  - out_W / out_b sharded over vocab (50257 -> pad 51200 -> 6400/core)
  - combine (1324->300) replicated on every core

All matvecs run on the TensorEngine with M=1 (batch) and the big weight
matrix as the *streaming* rhs operand, so weight bytes flow through the PE
at 1 col/cycle with only a 1-column LDWEIGHTS.  Biases are folded in as a
K=1 matmul with a ones lhsT.  Softmax / log-softmax use the online
max/sumexp merge: each core AllGathers (neg-max, sumexp) pairs (8 floats)
and applies the correction locally, so only tiny collectives sit on the
critical path.  Free-dim -> partition-dim relayouts go through small
SBUF->SBUF DMAs (PSUM banks stay disjoint; every PE instruction needs at
most 2 semaphore waits, which the MM instruction encoding requires).
"""

import sys

sys.path.insert(0, "/opt/trn_rl_repo")

import numpy as np

import concourse.bass as bass
import concourse.mybir as mybir
from concourse import bacc, tile

F32 = mybir.dt.float32
F32R = mybir.dt.float32r
BF16 = mybir.dt.bfloat16

NCORES = 8
I_SZ, H_SZ, V_SZ, L_SZ = 300, 1024, 50257, 4096
LSH = L_SZ // NCORES        # 512 seq positions per core
KA_T = 11                   # attn contraction tiles (11*128 = 1408 >= 1324)
VI = 6400                   # vocab shard per core (padded)
VPAD = VI * NCORES          # 51200
NCH = 13                    # stage-D chunks per core
CHS = [512] * 12 + [256]    # chunk widths (sum = 6400)
GS = 384                    # gate rows per core (3*128)
HS = H_SZ // NCORES         # 128 hidden units per core
NEG_BIG = -1.0e30
RG = [list(range(NCORES))]

MODE = "bf16"  # one of: f32, f32r, mixed, bf16


def _dtypes(mode):
    if mode == "bf16":
        return BF16, BF16
    if mode == "mixed":
        return F32, BF16
    return F32, F32


def build(mode=MODE):
    """Build the SPMD Bass program (same program on all 8 cores)."""
    adt, odt = _dtypes(mode)           # storage dtype: attn/gru weights, out_W
    use_f32r = mode in ("f32r", "mixed")

    def WA(ap):  # matmul-operand wrapper for the attn/gru path
        return ap.bitcast(F32R) if (use_f32r and adt == F32) else ap

    def WO(ap):  # matmul-operand wrapper for the output-projection path
        return ap.bitcast(F32R) if (use_f32r and odt == F32) else ap

    def cast_dma(out, in_, casting):
        # dtype casts must use SWDGE (gpsimd); everything else small goes on
        # the ACT HWDGE ring so it never queues behind the big weight DMAs
        if casting:
            nc.gpsimd.dma_start(out=out, in_=in_)
        else:
            nc.scalar.dma_start(out=out, in_=in_)

    nc = bacc.Bacc("TRN2", target_bir_lowering=False, debug=False,
                   num_devices=NCORES)

    # ---------------- external inputs (host pre-shaped / pre-permuted) ----
    d_vattn = nc.dram_tensor("v_attn", [128, KA_T], adt, kind="ExternalInput")
    d_aw = nc.dram_tensor("aw", [128, KA_T, LSH], adt, kind="ExternalInput")
    d_ab = nc.dram_tensor("ab", [LSH], adt, kind="ExternalInput")
    d_enc = nc.dram_tensor("enc", [128, 4, H_SZ], adt, kind="ExternalInput")
    d_emb = nc.dram_tensor("embk", [128, 3], adt, kind="ExternalInput")
    d_cw = nc.dram_tensor("cw", [128, KA_T, I_SZ], adt, kind="ExternalInput")
    d_cb = nc.dram_tensor("cb", [I_SZ], adt, kind="ExternalInput")
    d_h = nc.dram_tensor("h_full", [128, 8], adt, kind="ExternalInput")
    d_wih = nc.dram_tensor("wih", [128, 3, GS], adt, kind="ExternalInput")
    d_whh = nc.dram_tensor("whh", [128, 8, GS], adt, kind="ExternalInput")
    d_bih = nc.dram_tensor("bih", [GS], adt, kind="ExternalInput")
    d_bhh = nc.dram_tensor("bhh", [GS], adt, kind="ExternalInput")
    d_hmy = nc.dram_tensor("hmy", [HS], F32, kind="ExternalInput")
    d_wout_a = nc.dram_tensor("wout_a", [12, 128, 8, 512], odt,
                              kind="ExternalInput")
    d_wout_b = nc.dram_tensor("wout_b", [128, 8, 256], odt,
                              kind="ExternalInput")
    d_outb = nc.dram_tensor("outb", [VI], odt, kind="ExternalInput")

    # ---------------- external outputs ------------------------------------
    d_out_lp = nc.dram_tensor("out_logp", [NCH, 512], F32,
                              kind="ExternalOutput")
    d_out_h = nc.dram_tensor("out_h", [HS], F32, kind="ExternalOutput")
    d_out_aw = nc.dram_tensor("out_attnw", [LSH], F32, kind="ExternalOutput")

    X = mybir.AxisListType.X
    ADD = mybir.AluOpType.add
    MIN = mybir.AluOpType.min
    AF = mybir.ActivationFunctionType

    wout_bufs = 13 if (adt == BF16 and odt == BF16) else \
        (10 if odt == BF16 else 4)
    with tile.TileContext(nc) as tc:
        with (
            tc.tile_pool(name="w", bufs=1) as wp,
            tc.tile_pool(name="wout", bufs=wout_bufs) as wop,
            tc.tile_pool(name="ch", bufs=2) as chp,
            tc.tile_pool(name="ps", bufs=1, space="PSUM") as pp,
            tc.tile_pool(name="dram", bufs=1, space="DRAM") as dp,
        ):
            # ---- collective bounce buffers in DRAM ----
            cc_a_in = dp.tile([8], F32, name="cc_a_in")
            cc_a_out = dp.tile([8 * NCORES], F32, addr_space="Shared",
                               name="cc_a_out")
            cc_b_in = dp.tile([H_SZ], F32, name="cc_b_in")
            cc_b_out = dp.tile([H_SZ], F32, addr_space="Shared",
                               name="cc_b_out")
            cc_h_in = dp.tile([HS], F32, name="cc_h_in")
            cc_h_out = dp.tile([H_SZ], F32, addr_space="Shared",
                               name="cc_h_out")
            cc_d_in = dp.tile([8], F32, name="cc_d_in")
            cc_d_out = dp.tile([8 * NCORES], F32, addr_space="Shared",
                               name="cc_d_out")

            # ---- constants ----
            ones_a = wp.tile([1, 1], adt, name="ones_a")
            nc.vector.memset(ones_a[:, :], 1.0)
            ones_o = wp.tile([1, 1], odt, name="ones_o")
            nc.vector.memset(ones_o[:, :], 1.0)
            ones13 = wp.tile([1, NCH], F32, name="ones13")
            nc.vector.memset(ones13[:, :], 1.0)
            logits = wp.tile([NCH, 512], F32, name="logits")
            nc.vector.memset(logits[:, :], NEG_BIG)
            x_sb = wp.tile([1, 384], F32, name="x_sb")
            nc.vector.memset(x_sb[:, :], 0.0)

            # ---- weight / vector loads (priority order) ----
            vk = wp.tile([128, KA_T], adt, name="vk")
            nc.sync.dma_start(out=vk[:, :], in_=d_vattn[:, :])
            aw_sb = wp.tile([128, KA_T, LSH], adt, name="aw_sb")
            nc.sync.dma_start(out=aw_sb[:, :, :], in_=d_aw[:, :, :])
            ab_sb = wp.tile([1, LSH], adt, name="ab_sb")
            nc.sync.dma_start(out=ab_sb[0:1, :], in_=d_ab[:])
            enc_sb = wp.tile([128, 4, H_SZ], adt, name="enc_sb")
            nc.sync.dma_start(out=enc_sb[:, :, :], in_=d_enc[:, :, :])
            hk = wp.tile([128, 8], adt, name="hk")
            nc.sync.dma_start(out=hk[:, :], in_=d_h[:, :])
            whh_sb = wp.tile([128, 8, GS], adt, name="whh_sb")
            nc.sync.dma_start(out=whh_sb[:, :, :], in_=d_whh[:, :, :])
            bhh_sb = wp.tile([1, GS], adt, name="bhh_sb")
            nc.sync.dma_start(out=bhh_sb[0:1, :], in_=d_bhh[:])
            comb_e = wp.tile([128, 3], adt, name="comb_e")
            nc.sync.dma_start(out=comb_e[:, :], in_=d_emb[:, :])
            cw_sb = wp.tile([128, KA_T, I_SZ], adt, name="cw_sb")
            nc.sync.dma_start(out=cw_sb[:, :, :], in_=d_cw[:, :, :])
            cb_sb = wp.tile([1, I_SZ], adt, name="cb_sb")
            nc.sync.dma_start(out=cb_sb[0:1, :], in_=d_cb[:])
            wih_sb = wp.tile([128, 3, GS], adt, name="wih_sb")
            nc.sync.dma_start(out=wih_sb[:, :, :], in_=d_wih[:, :, :])
            bih_sb = wp.tile([1, GS], adt, name="bih_sb")
            nc.sync.dma_start(out=bih_sb[0:1, :], in_=d_bih[:])
            hmy_sb = wp.tile([1, HS], F32, name="hmy_sb")
            nc.sync.dma_start(out=hmy_sb[0:1, :], in_=d_hmy[:])
            outb_sb = wp.tile([1, VI], odt, name="outb_sb")
            nc.sync.dma_start(out=outb_sb[0:1, :], in_=d_outb[:])

            # ============= stage A: attention scores + local stats ========
            ps_sc = pp.tile([1, LSH], F32, name="ps_sc")
            for t in range(KA_T):
                nc.tensor.matmul(ps_sc[0:1, :], WA(vk[:, t:t + 1]),
                                 WA(aw_sb[:, t, :]),
                                 start=(t == 0), stop=False)
            nc.tensor.matmul(ps_sc[0:1, :], WA(ones_a[:, :]),
                             WA(ab_sb[0:1, :]), start=False, stop=True)

            nm_a = wp.tile([1, 1], F32, name="nm_a")      # -max(scores)
            nc.vector.reduce_max(nm_a[:, :], ps_sc[0:1, :], X, negate=True)
            e_loc = wp.tile([1, LSH], F32, name="e_loc")
            s_a = wp.tile([1, 1], F32, name="s_a")
            nc.scalar.activation(e_loc[0:1, :], ps_sc[0:1, :], AF.Exp,
                                 bias=nm_a[:, :], scale=1.0,
                                 accum_out=s_a[:, :])
            ms_a = wp.tile([1, 8], F32, name="ms_a")
            nc.vector.memset(ms_a[:, :], 0.0)
            nc.vector.tensor_copy(ms_a[0:1, 0:1], nm_a[:, :])
            nc.vector.tensor_copy(ms_a[0:1, 1:2], s_a[:, :])
            nc.scalar.dma_start(out=cc_a_in[:], in_=ms_a[0:1, :])
            nc.gpsimd.collective_compute(
                "AllGather", mybir.AluOpType.bypass, replica_groups=RG,
                ins=[cc_a_in[:]], outs=[cc_a_out[:]])

            # ---- relayout e_loc to partition layout (overlaps AG#1) ------
            ek = wp.tile([128, 4], adt, name="ek")
            cast_dma(ek[:, :], e_loc[0:1, :], adt != F32)

            # ============= stage B: partial attn_applied ==================
            ps_att = pp.tile([1, H_SZ], F32, name="ps_att")
            for nb in range(2):
                sl = slice(nb * 512, (nb + 1) * 512)
                for t in range(4):
                    nc.tensor.matmul(ps_att[0:1, sl], WA(ek[:, t:t + 1]),
                                     WA(enc_sb[:, t, sl]),
                                     start=(t == 0), stop=(t == 3))

            # ---- gh = h @ W_hh.T + b_hh (PE busywork during AG/AR) -------
            ps_gh = pp.tile([1, GS], F32, name="ps_gh")
            for t in range(8):
                nc.tensor.matmul(ps_gh[0:1, :], WA(hk[:, t:t + 1]),
                                 WA(whh_sb[:, t, :]),
                                 start=(t == 0), stop=False)
            nc.tensor.matmul(ps_gh[0:1, :], WA(ones_a[:, :]),
                             WA(bhh_sb[0:1, :]), start=False, stop=True)
            gh_sb = wp.tile([1, GS], F32, name="gh_sb")
            nc.scalar.copy(gh_sb[0:1, :], ps_gh[0:1, :])

            # ---- AG#1 result: merge global max / sumexp ------------------
            ms8_a = wp.tile([1, 8 * NCORES], F32, name="ms8_a")
            nc.scalar.dma_start(out=ms8_a[0:1, :], in_=cc_a_out[:])
            ms8_av = ms8_a.rearrange("p (r k) -> p r k", k=8)
            nmG_a = wp.tile([1, 1], F32, name="nmG_a")   # -global max
            nc.vector.tensor_reduce(nmG_a[:, :], ms8_av[:, :, 0], X, MIN)
            corr_a = wp.tile([1, NCORES], F32, name="corr_a")
            nc.scalar.activation(corr_a[0:1, :], ms8_av[:, :, 0], AF.Exp,
                                 bias=nmG_a[:, :], scale=-1.0)
            sc_a = wp.tile([1, NCORES], F32, name="sc_a")
            nc.vector.tensor_mul(sc_a[0:1, :], corr_a[0:1, :],
                                 ms8_av[:, :, 1])
            S_a = wp.tile([1, 1], F32, name="S_a")
            nc.vector.tensor_reduce(S_a[:, :], sc_a[0:1, :], X, ADD)
            rS_a = wp.tile([1, 1], F32, name="rS_a")
            nc.vector.reciprocal(rS_a[:, :], S_a[:, :])
            cme_a = wp.tile([1, 1], F32, name="cme_a")   # exp(m_c - M)
            nc.scalar.activation(cme_a[0:1, :], nm_a[0:1, :], AF.Exp,
                                 bias=nmG_a[:, :], scale=-1.0)
            scme = wp.tile([1, 1], F32, name="scme")     # exp(m_c-M)/S
            nc.vector.tensor_mul(scme[:, :], cme_a[:, :], rS_a[:, :])

            # local attention weights output slice
            w_loc = wp.tile([1, LSH], F32, name="w_loc")
            nc.scalar.activation(w_loc[0:1, :], e_loc[0:1, :], AF.Copy,
                                 bias=0.0, scale=scme[:, :])
            nc.scalar.dma_start(out=d_out_aw[:], in_=w_loc[0:1, :])

            # scaled partial attn_applied -> AllReduce
            attp = wp.tile([1, H_SZ], F32, name="attp")
            nc.scalar.activation(attp[0:1, :], ps_att[0:1, :], AF.Copy,
                                 bias=0.0, scale=scme[:, :])
            nc.scalar.dma_start(out=cc_b_in[:], in_=attp[0:1, :])
            nc.gpsimd.collective_compute(
                "AllReduce", ADD, replica_groups=RG,
                ins=[cc_b_in[:]], outs=[cc_b_out[:]])

            # ============= stage C: combine + GRU =========================
            comb_a = wp.tile([128, 8], adt, name="comb_a")
            cast_dma(comb_a[:, :], cc_b_out[:], adt != F32)

            ps_x = pp.tile([1, I_SZ], F32, name="ps_x")
            for t in range(3):
                nc.tensor.matmul(ps_x[0:1, :], WA(comb_e[:, t:t + 1]),
                                 WA(cw_sb[:, t, :]),
                                 start=(t == 0), stop=False)
            for t in range(3, KA_T):
                nc.tensor.matmul(ps_x[0:1, :], WA(comb_a[:, t - 3:t - 2]),
                                 WA(cw_sb[:, t, :]),
                                 start=False, stop=False)
            nc.tensor.matmul(ps_x[0:1, :], WA(ones_a[:, :]),
                             WA(cb_sb[0:1, :]), start=False, stop=True)
            nc.scalar.activation(x_sb[0:1, 0:I_SZ], ps_x[0:1, :], AF.Relu)

            xk = wp.tile([128, 3], adt, name="xk")
            cast_dma(xk[:, :], x_sb[0:1, :], adt != F32)

            ps_gi = pp.tile([1, GS], F32, name="ps_gi")
            for t in range(3):
                nc.tensor.matmul(ps_gi[0:1, :], WA(xk[:, t:t + 1]),
                                 WA(wih_sb[:, t, :]),
                                 start=(t == 0), stop=False)
            nc.tensor.matmul(ps_gi[0:1, :], WA(ones_a[:, :]),
                             WA(bih_sb[0:1, :]), start=False, stop=True)

            # gates: r,z = sigmoid(gi+gh)[0:256]; n = tanh(gi_n + r*gh_n)
            rz_in = wp.tile([1, 256], F32, name="rz_in")
            nc.vector.tensor_add(rz_in[0:1, :], ps_gi[0:1, 0:256],
                                 gh_sb[0:1, 0:256])
            rz = wp.tile([1, 256], F32, name="rz")
            nc.scalar.activation(rz[0:1, :], rz_in[0:1, :], AF.Sigmoid)
            rn = wp.tile([1, HS], F32, name="rn")
            nc.vector.tensor_mul(rn[0:1, :], rz[0:1, 0:HS],
                                 gh_sb[0:1, 256:GS])
            n_in = wp.tile([1, HS], F32, name="n_in")
            nc.vector.tensor_add(n_in[0:1, :], ps_gi[0:1, 256:GS],
                                 rn[0:1, :])
            n_t = wp.tile([1, HS], F32, name="n_t")
            nc.scalar.activation(n_t[0:1, :], n_in[0:1, :], AF.Tanh)
            d_tl = wp.tile([1, HS], F32, name="d_tl")
            nc.vector.tensor_sub(d_tl[0:1, :], hmy_sb[0:1, :], n_t[0:1, :])
            zd = wp.tile([1, HS], F32, name="zd")
            nc.vector.tensor_mul(zd[0:1, :], rz[0:1, HS:256], d_tl[0:1, :])
            hn_new = wp.tile([1, HS], F32, name="hn_new")
            nc.vector.tensor_add(hn_new[0:1, :], n_t[0:1, :], zd[0:1, :])

            nc.scalar.dma_start(out=d_out_h[:], in_=hn_new[0:1, :])
            nc.scalar.dma_start(out=cc_h_in[:], in_=hn_new[0:1, :])
            nc.gpsimd.collective_compute(
                "AllGather", mybir.AluOpType.bypass, replica_groups=RG,
                ins=[cc_h_in[:]], outs=[cc_h_out[:]])

            hnk = wp.tile([128, 8], odt, name="hnk")
            cast_dma(hnk[:, :], cc_h_out[:], odt != F32)

            # ============= stage D: logits + log_softmax ==================
            for j in range(NCH):
                n_j = CHS[j]
                wt = wop.tile([128, 8, 512], odt, tag="wt", name="wt")
                if j < 12:
                    nc.sync.dma_start(out=wt[:, :, :],
                                      in_=d_wout_a[j, :, :, :])
                else:
                    nc.sync.dma_start(out=wt[:, :, 0:256],
                                      in_=d_wout_b[:, :, :])
                ps_d = pp.tile([1, 512], F32, tag="ps_d", name="ps_d",
                               bufs=2)
                for t in range(8):
                    nc.tensor.matmul(ps_d[0:1, 0:n_j],
                                     WO(hnk[:, t:t + 1]),
                                     WO(wt[:, t, 0:n_j]),
                                     start=(t == 0), stop=False)
                nc.tensor.matmul(ps_d[0:1, 0:n_j], WO(ones_o[:, :]),
                                 WO(outb_sb[0:1, j * 512:j * 512 + n_j]),
                                 start=False, stop=True)
                ch = chp.tile([1, 512], F32, tag="ch", name="ch")
                nc.scalar.copy(ch[0:1, 0:n_j], ps_d[0:1, 0:n_j])
                nc.scalar.dma_start(out=logits[j:j + 1, 0:n_j],
                                  in_=ch[0:1, 0:n_j])

            # ---- local log-softmax stats over [13, 512] ------------------
            nm_d = wp.tile([NCH, 1], F32, name="nm_d")   # -rowmax
            nc.vector.reduce_max(nm_d[:, :], logits[:, :], X, negate=True)
            e_d = wp.tile([NCH, 512], F32, name="e_d")
            s_d = wp.tile([NCH, 1], F32, name="s_d")
            nc.scalar.activation(e_d[:, :], logits[:, :], AF.Exp,
                                 bias=nm_d[:, :], scale=1.0,
                                 accum_out=s_d[:, :])
            mT = wp.tile([1, NCH], F32, name="mT")
            nc.scalar.dma_start(out=mT[0:1, :], in_=nm_d[:, 0:1])
            sT = wp.tile([1, NCH], F32, name="sT")
            nc.scalar.dma_start(out=sT[0:1, :], in_=s_d[:, 0:1])
            nmL = wp.tile([1, 1], F32, name="nmL")       # -local max
            nc.vector.tensor_reduce(nmL[:, :], mT[0:1, :], X, MIN)
            corr_d = wp.tile([1, NCH], F32, name="corr_d")
            nc.scalar.activation(corr_d[0:1, :], mT[0:1, :], AF.Exp,
                                 bias=nmL[:, :], scale=-1.0)
            scd = wp.tile([1, NCH], F32, name="scd")
            nc.vector.tensor_mul(scd[0:1, :], corr_d[0:1, :], sT[0:1, :])
            S_dl = wp.tile([1, 1], F32, name="S_dl")
            nc.vector.tensor_reduce(S_dl[:, :], scd[0:1, :], X, ADD)

            ms_d = wp.tile([1, 8], F32, name="ms_d")
            nc.vector.memset(ms_d[:, :], 0.0)
            nc.vector.tensor_copy(ms_d[0:1, 0:1], nmL[:, :])
            nc.vector.tensor_copy(ms_d[0:1, 1:2], S_dl[:, :])
            nc.scalar.dma_start(out=cc_d_in[:], in_=ms_d[0:1, :])
            nc.gpsimd.collective_compute(
                "AllGather", mybir.AluOpType.bypass, replica_groups=RG,
                ins=[cc_d_in[:]], outs=[cc_d_out[:]])

            ms8_d = wp.tile([1, 8 * NCORES], F32, name="ms8_d")
            nc.scalar.dma_start(out=ms8_d[0:1, :], in_=cc_d_out[:])
            ms8_dv = ms8_d.rearrange("p (r k) -> p r k", k=8)
            nmG_d = wp.tile([1, 1], F32, name="nmG_d")   # -global max
            nc.vector.tensor_reduce(nmG_d[:, :], ms8_dv[:, :, 0], X, MIN)
            corr_g = wp.tile([1, NCORES], F32, name="corr_g")
            nc.scalar.activation(corr_g[0:1, :], ms8_dv[:, :, 0], AF.Exp,
                                 bias=nmG_d[:, :], scale=-1.0)
            sc_g = wp.tile([1, NCORES], F32, name="sc_g")
            nc.vector.tensor_mul(sc_g[0:1, :], corr_g[0:1, :],
                                 ms8_dv[:, :, 1])
            S_g = wp.tile([1, 1], F32, name="S_g")
            nc.vector.tensor_reduce(S_g[:, :], sc_g[0:1, :], X, ADD)
            lnS = wp.tile([1, 1], F32, name="lnS")
            nc.scalar.activation(lnS[0:1, :], S_g[0:1, :], AF.Ln)
            nshift = wp.tile([1, 1], F32, name="nshift")  # -(M + ln S)
            nc.vector.tensor_sub(nshift[:, :], nmG_d[:, :], lnS[:, :])
            nsh13 = wp.tile([1, NCH], F32, name="nsh13")
            nc.vector.tensor_scalar_mul(nsh13[0:1, :], ones13[0:1, :],
                                        nshift[:, :])
            nb13 = wp.tile([NCH, 1], F32, name="nb13")
            nc.scalar.dma_start(out=nb13[:, 0:1], in_=nsh13[0:1, :])

            outlp = wp.tile([NCH, 512], F32, name="outlp")
            nc.scalar.activation(outlp[:, :], logits[:, :], AF.Identity,
                                 bias=nb13[:, :], scale=1.0)
            nc.scalar.dma_start(out=d_out_lp[:, :], in_=outlp[:, :])

    nc.compile()
    return nc


def prepare_in_maps(embedded, hidden, encoder_outputs, attn_W, attn_b,
                    combine_W, combine_b, W_ih, W_hh, b_ih, b_hh, out_W,
                    out_b, mode=MODE):
    adt, odt = _dtypes(mode)
    anp = mybir.dt.np(adt)
    onp = mybir.dt.np(odt)

    f32 = np.float32
    emb = np.asarray(embedded, f32).reshape(I_SZ)
    h0 = np.asarray(hidden, f32).reshape(H_SZ)
    enc = np.asarray(encoder_outputs, f32)
    aW = np.asarray(attn_W, f32)
    ab = np.asarray(attn_b, f32)
    cW = np.asarray(combine_W, f32)
    cb = np.asarray(combine_b, f32)
    Wih = np.asarray(W_ih, f32)
    Whh = np.asarray(W_hh, f32)
    bih = np.asarray(b_ih, f32)
    bhh = np.asarray(b_hh, f32)
    oW = np.asarray(out_W, f32)
    ob = np.asarray(out_b, f32)

    v = np.zeros(128 * KA_T, f32)
    v[:I_SZ] = emb
    v[I_SZ:I_SZ + H_SZ] = h0
    v_attn = v.reshape(128, KA_T).astype(anp)

    emb_pad = np.zeros(384, f32)
    emb_pad[:I_SZ] = emb
    embk = emb_pad.reshape(128, 3).astype(anp)
    h_full = h0.reshape(128, 8).astype(anp)

    AWT = aW.T  # [1324, 4096]
    CWT = cW.T  # [1324, 300]
    cw_e = np.zeros((384, I_SZ), f32)
    cw_e[:I_SZ] = CWT[:I_SZ]
    cw_host = np.concatenate(
        [cw_e.reshape(128, 3, I_SZ), CWT[I_SZ:].reshape(128, 8, I_SZ)],
        axis=1).astype(anp)  # [128, 11, 300]

    WIHT = Wih.T  # [300, 3072]
    WHHT = Whh.T  # [1024, 3072]

    WTp = np.zeros((H_SZ, VPAD), f32)
    WTp[:, :V_SZ] = oW.T
    obp = np.full(VPAD, NEG_BIG, f32)
    obp[:V_SZ] = ob

    in_maps = []
    for c in range(NCORES):
        AWc = np.zeros((128 * KA_T, LSH), f32)
        AWc[:I_SZ + H_SZ] = AWT[:, c * LSH:(c + 1) * LSH]
        gidx = np.concatenate([np.arange(c * HS, (c + 1) * HS) + k * H_SZ
                               for k in range(3)])
        wih_p = np.zeros((384, GS), f32)
        wih_p[:I_SZ] = WIHT[:, gidx]
        Wc = WTp[:, c * VI:(c + 1) * VI].reshape(128, 8, VI)
        in_maps.append({
            "v_attn": v_attn,
            "aw": AWc.reshape(128, KA_T, LSH).astype(anp),
            "ab": ab[c * LSH:(c + 1) * LSH].astype(anp),
            "enc": enc[c * LSH:(c + 1) * LSH].reshape(128, 4, H_SZ)
                   .astype(anp).copy(),
            "embk": embk,
            "cw": cw_host,
            "cb": cb.astype(anp),
            "h_full": h_full,
            "wih": wih_p.reshape(128, 3, GS).astype(anp).copy(),
            "whh": WHHT[:, gidx].reshape(128, 8, GS).astype(anp),
            "bih": bih[gidx].astype(anp),
            "bhh": bhh[gidx].astype(anp),
            "hmy": h0[c * HS:(c + 1) * HS],
            "wout_a": Wc[:, :, :6144].reshape(128, 8, 12, 512)
                      .transpose(2, 0, 1, 3).astype(onp).copy(),
            "wout_b": Wc[:, :, 6144:].astype(onp).copy(),
            "outb": obp[c * VI:(c + 1) * VI].astype(onp),
        })
    return in_maps


def gather_outputs(results):
    """results: list of 8 dicts with out_logp/out_h/out_attnw."""
    lp_parts = []
    for c in range(NCORES):
        r = np.asarray(results[c]["out_logp"], np.float32).reshape(NCH, 512)
        lp_parts.append(r[:12].reshape(-1))
        lp_parts.append(r[12, :256])
    output = np.concatenate(lp_parts)[:V_SZ][None, :]
    h_new = np.concatenate(
        [np.asarray(results[c]["out_h"], np.float32).reshape(-1)
         for c in range(NCORES)])[None, None, :]
    attn_w = np.concatenate(
        [np.asarray(results[c]["out_attnw"], np.float32).reshape(-1)
         for c in range(NCORES)])[None, :]
    return output, h_new, attn_w


_NC_CACHE = {}


def kernel(embedded, hidden, encoder_outputs, attn_W, attn_b,
           combine_W, combine_b, W_ih, W_hh, b_ih, b_hh, out_W, out_b):
    from concourse.bass_utils import run_bass_kernel_spmd

    if MODE not in _NC_CACHE:
        _NC_CACHE[MODE] = build(MODE)
    nc = _NC_CACHE[MODE]
    in_maps = prepare_in_maps(embedded, hidden, encoder_outputs, attn_W,
                              attn_b, combine_W, combine_b, W_ih, W_hh,
                              b_ih, b_hh, out_W, out_b, mode=MODE)
    res = run_bass_kernel_spmd(nc, in_maps, list(range(NCORES)))
    return gather_outputs(res.results)
